# revision 28
# baseline (speedup 1.0000x reference)
"""BasicTransformerBlock Trainium2 kernel, v3.

Sharding: 8 cores = 2 batch groups x 4 sequence shards. The host rotates each
core's rows so its own 512 rows are always rows 0..511 (pure SPMD, no
collectives). Attention is key-order invariant, so each core recomputes
adaLN1 + K/V projections over the full (rotated) 2048-row sequence of its
batch; Q/attention/out-proj/FFN are local to its own 512 rows.

Key differences vs v2:
- AdaLN scale/shift folded into the weights on HOST:
    adaln(x) @ W = LN(x) @ (diag(1+s) W) + shift @ W
  so the device only computes plain LN (stats + per-partition normalize),
  transpose, and a raw copy-evict. K-projection bias is dropped entirely
  (additive per-query constants cancel in softmax); V bias folds through
  the out-projection into bo; attn1's bo folds into the residual rows
  shipped from host; Q bias is a per-partition scalar in the QT evict.
- Softmax tail rebuilt: denominators accumulate into one [1,2,512] PSUM
  row pair, eviction is one small DVE copy, partition-broadcast is a K=1
  rank-1 PE matmul, reciprocal is the fast custom-DVE approx on all 128
  partitions. No more 4us single-lane reciprocals / gpsimd broadcast, so
  the PE never idles long enough for HAM to re-throttle.
- attn2/FFN biases (bo2, b2) enter the PSUM accumulation as K=1 ones-row
  matmuls instead of extra DVE passes.
- FFN gT@W2 matmuls lag one iteration behind the W1 matmuls so the
  in-order PE queue never head-of-line blocks on the gelu/GLU chain.
- Phase 1a is chunk-interleaved: adaLN of 512 rows, then K/V projections
  of those rows, keeping the PE dense from the start.

Heavy matmuls in bf16 with fp32 PSUM accumulation.
"""

import numpy as np
import ml_dtypes

import concourse.bass as bass
import concourse.bacc as bacc
import concourse.mybir as mybir
import concourse.tile as tile
from concourse import bass_utils
from concourse.masks import make_identity

P = 128
B, S, CTX, D, H, DH = 2, 2048, 256, 1024, 16, 64
INNER = H * DH          # 1024
DFF = 4 * D             # 4096
NCORES = 8
OWN = 512               # rows owned per core
NPAIR = H // 2          # 8 head pairs
DB = D // P             # 8 model-dim blocks
F32 = mybir.dt.float32
BF16 = mybir.dt.bfloat16
NPBF16 = ml_dtypes.bfloat16

AF = mybir.ActivationFunctionType
ALU = mybir.AluOpType


def _adaln_stats(nc, pools, x_src, name):
    """LN stats for one 128-row tile: DVE/ScalarE only, no PE. Returns the
    (nmr, rstd) per-partition scalars for the later normalize pass."""
    wk = pools["wk"]
    stats = wk.tile([P, 2, 6], F32, name=f"st_{name}", tag="stats", bufs=5)
    nc.vector.bn_stats(stats[:, 0, :], x_src[:, 0:512])
    nc.vector.bn_stats(stats[:, 1, :], x_src[:, 512:1024])
    mv = wk.tile([P, 2], F32, name=f"mv_{name}", tag="mv", bufs=5)
    nc.vector.bn_aggr(mv, stats)
    sd = wk.tile([P, 1], F32, name=f"sd_{name}", tag="sd", bufs=5)
    nc.scalar.activation(sd, mv[:, 1:2], AF.Sqrt, bias=pools["eps"][:, 0:1])
    rstd = wk.tile([P, 1], F32, name=f"rs_{name}", tag="rstd", bufs=5)
    nc.vector.reciprocal(rstd, sd)
    nmr = wk.tile([P, 1], F32, name=f"nm_{name}", tag="nmr", bufs=5)
    nc.vector.tensor_scalar(nmr, mv[:, 0:1], rstd, -1.0,
                            op0=ALU.mult, op1=ALU.mult)
    return nmr, rstd


def _adaln_trans(nc, pools, x_src, st, hT_dst, tr_pool, name):
    """Normalize (per-partition affine on ScalarE) + transpose + evict."""
    wk = pools["wk"]
    nmr, rstd = st
    xn = wk.tile([P, D], BF16, name=f"xn_{name}", tag="xn", bufs=2)
    nc.scalar.activation(xn, x_src, AF.Identity, bias=nmr[:, 0:1],
                         scale=rstd[:, 0:1])
    for db in range(DB):
        ps_t = tr_pool.tile([P, P], BF16, name=f"pt_{name}_{db}", tag="tr",
                            bufs=4)
        nc.tensor.transpose(ps_t, xn[:, db * P:(db + 1) * P], pools["idt"])
        if db % 2 == 0:
            nc.scalar.copy(hT_dst(db), ps_t)
        else:
            nc.vector.tensor_copy(hT_dst(db), ps_t)


def _adaln_tile(nc, pools, x_src, hT_dst, tr_pool, name):
    st = _adaln_stats(nc, pools, x_src, name)
    _adaln_trans(nc, pools, x_src, st, hT_dst, tr_pool, name)


def _mha_core(nc, pools, KT, VT, QT, n_kb, ps_pool, wo_sb, bias_row,
              x_dst_write, name):
    """Attention core + out-projection (+ optional bias matmul) + residual.

    The kb loop is software-pipelined: scores/exp for step kb issue before
    the PV/denominator matmuls of step kb-1. Softmax normalization:
    denominators for both heads accumulate into one [1,2,512] PSUM row,
    get evicted bf16, rank-1 broadcast across partitions on the PE, and
    fast-approx reciprocated on the DVE over all 128 lanes.

    KT: [128, 8, n_kb*128] bf16 (pair-dim on partitions, keys on free)
    VT: [128, n_kb, 1024] bf16  (key rows on partitions, inner on free)
    QT: [128, 8, 512] bf16
    wo_sb: [128, 8, 1024] bf16 resident out-proj weights
    bias_row: [1, D] bf16 SBUF row added via K=1 matmul, or None
    x_dst_write(rc, half, ps_ap): consume finished out-proj PSUM chunk
    """
    wk = pools["wk"]
    outT = pools["outT"]
    ones = pools["ones"]          # [P, 1] bf16: K=128, M=1 (denominator)
    ones_row = pools["ones_row"]  # [1, P] bf16: K=1, M=128 (broadcast)

    for hp in range(NPAIR):
        pv = ps_pool.tile([P, 512], F32, name=f"pv_{name}_{hp}", tag="pv",
                          bufs=2)
        dn = ps_pool.tile([P, 2, 512], F32, name=f"dn_{name}_{hp}", tag="dn",
                          bufs=1)
        probs_q = {}

        def scores_step(kb):
            ps_s = ps_pool.tile([P, 2, 512], F32, name=f"s_{name}_{hp}_{kb}",
                                tag="sc", bufs=2)
            nc.tensor.matmul(ps_s[:, 0, :], KT[0:64, hp, kb * P:(kb + 1) * P],
                             QT[0:64, hp, :], start=True, stop=True)
            nc.tensor.matmul(ps_s[:, 1, :], KT[64:128, hp, kb * P:(kb + 1) * P],
                             QT[64:128, hp, :], start=True, stop=True,
                             tile_position=(64, 0))
            probs = wk.tile([P, 2, 512], BF16, name=f"pr_{name}_{hp}_{kb}",
                            tag="probs", bufs=3)
            nc.scalar.activation(probs, ps_s, AF.Exp, scale=0.125)
            probs_q[kb] = probs

        def pv_step(kb):
            probs = probs_q.pop(kb)
            nc.tensor.matmul(pv[0:64, :], VT[:, kb, hp * P:hp * P + 64],
                             probs[:, 0, :], start=(kb == 0),
                             stop=(kb == n_kb - 1))
            nc.tensor.matmul(pv[64:128, :],
                             VT[:, kb, hp * P + 64:hp * P + 128],
                             probs[:, 1, :], start=(kb == 0),
                             stop=(kb == n_kb - 1), tile_position=(0, 64))
            nc.tensor.matmul(dn[0:1, 0, :], ones, probs[:, 0, :],
                             start=(kb == 0), stop=(kb == n_kb - 1))
            nc.tensor.matmul(dn[0:1, 1, :], ones, probs[:, 1, :],
                             start=(kb == 0), stop=(kb == n_kb - 1))

        lead = 2 if n_kb > 2 else 1
        for kb in range(n_kb):
            scores_step(kb)
            if kb >= lead:
                pv_step(kb - lead)
        for kb in range(n_kb - lead, n_kb):
            pv_step(kb)

        dn_sb = wk.tile([1, 2, 512], BF16, name=f"dsb_{name}_{hp}",
                        tag="dnsb", bufs=1)
        nc.vector.tensor_copy(dn_sb, dn[0:1, :, :])
        bc = ps_pool.tile([P, 512], F32, name=f"bc_{name}_{hp}", tag="sc",
                          bufs=2)
        nc.tensor.matmul(bc[0:64, :], ones_row[0:1, 0:64], dn_sb[0:1, 0, :],
                         start=True, stop=True)
        nc.tensor.matmul(bc[64:128, :], ones_row[0:1, 0:64],
                         dn_sb[0:1, 1, :], start=True, stop=True,
                         tile_position=(0, 64))
        rec = wk.tile([P, 512], F32, name=f"rec_{name}_{hp}", tag="rec",
                      bufs=1)
        nc.vector.reciprocal_approx_fast(rec, bc)
        nc.vector.tensor_tensor(outT[0:64, hp, :], pv[0:64, :],
                                rec[0:64, :], op=ALU.mult)
        nc.vector.tensor_tensor(outT[64:128, hp, :], pv[64:128, :],
                                rec[64:128, :], op=ALU.mult)

    # out-projection (+bias via K=1 matmul) + residual, row-chunk outer so
    # the residual stream completes chunk-by-chunk.
    for rc in range(4):
        for half in range(2):
            ps = ps_pool.tile([P, 512], F32, name=f"op_{name}_{rc}_{half}",
                              tag="sc", bufs=2)
            if bias_row is not None:
                nc.tensor.matmul(ps, ones_row,
                                 bias_row[0:1, half * 512:(half + 1) * 512],
                                 start=True, stop=False)
            for hp in range(NPAIR):
                nc.tensor.matmul(ps, outT[:, hp, rc * P:(rc + 1) * P],
                                 wo_sb[:, hp, half * 512:(half + 1) * 512],
                                 start=(hp == 0 and bias_row is None),
                                 stop=(hp == NPAIR - 1))
            x_dst_write(rc, half, ps)


def build_program():
    nc = bacc.Bacc("TRN2", target_bir_lowering=False, debug=False,
                   num_devices=NCORES)
    d = {}

    def din(nm, shape, dt):
        d[nm] = nc.dram_tensor(nm, shape, dt, kind="ExternalInput").ap()
        return d[nm]

    din("x_bf", [S, D], BF16)          # full rotated rows, LN input
    din("x_own", [OWN, D], F32)        # own rows + bo1 folded, residual
    din("ctx", [CTX, D], BF16)
    din("bq1", [P, DB], F32)           # Q1 bias, feature-major columns
    din("bq2", [P, DB], F32)
    for a in ("a1", "a2"):
        din(f"{a}_wqT", [DB, P, DB, P], BF16)   # [ib, p, db, j]
        din(f"{a}_wkT", [DB, P, DB, P] if a == "a2" else [P, DB, DB, P],
            BF16)
        din(f"{a}_wv", [DB, P, INNER], BF16)    # [db, p, j]
        din(f"{a}_wo", [NPAIR, P, D], BF16)     # [hp, p, j]
    din("bo2", [1, D], BF16)
    din("w1", [64, P, DB, P], BF16)             # [chunk, p, db, j]
    din("b1a", [P, 32], F32)
    din("b1g", [P, 32], F32)
    din("w2", [32, P, D], BF16)                 # [kb, p, j]
    din("b2", [1, D], BF16)
    out_d = nc.dram_tensor("out", [OWN, D], F32, kind="ExternalOutput").ap()

    with tile.TileContext(nc) as tc:
        import contextlib
        with contextlib.ExitStack() as ctx:
            const = ctx.enter_context(tc.tile_pool(name="const", bufs=1))
            persist = ctx.enter_context(tc.tile_pool(name="persist", bufs=1))
            wk = ctx.enter_context(tc.tile_pool(name="wkp", bufs=1))

            pools = {"wk": wk}
            xpre = []
            for ti in range(3):
                x_t = wk.tile([P, D], BF16, name=f"x1_{ti}", tag="xg", bufs=3)
                nc.sync.dma_start(x_t, d["x_bf"][ti * P:(ti + 1) * P, :])
                xpre.append(x_t)
            idt = const.tile([P, P], BF16, name="idt")
            make_identity(nc, idt)
            pools["idt"] = idt
            ones_bf = const.tile([P, 1], BF16, name="ones_bf")
            nc.vector.memset(ones_bf, 1.0)
            pools["ones"] = ones_bf
            ones_row = const.tile([1, P], BF16, name="ones_row")
            nc.vector.memset(ones_row, 1.0)
            pools["ones_row"] = ones_row
            eps_t = const.tile([P, 1], F32, name="eps_t")
            nc.vector.memset(eps_t, 1e-5)
            pools["eps"] = eps_t
            bq1_sb = const.tile([P, DB], F32, name="bq1_sb")
            nc.sync.dma_start(bq1_sb, d["bq1"])
            bq2_sb = const.tile([P, DB], F32, name="bq2_sb")
            nc.sync.dma_start(bq2_sb, d["bq2"])

            K1T = persist.tile([P, NPAIR, S], BF16, name="K1T", tag="K1T")
            V1 = persist.tile([P, S // P, INNER], BF16, name="V1", tag="V1")
            Q1T = persist.tile([P, NPAIR, OWN], BF16, name="Q1T", tag="qT",
                               bufs=1)
            outT = persist.tile([P, NPAIR, OWN], BF16, name="outT", tag="outT")
            pools["outT"] = outT
            K2T = persist.tile([P, NPAIR, CTX], BF16, name="K2T", tag="K2T")
            V2 = persist.tile([P, CTX // P, INNER], BF16, name="V2", tag="V2")
            x1s = persist.tile([P, 4, D], F32, name="x1s", tag="x1s")
            hT = persist.tile([P, DB, 2, OWN], BF16, name="hT", tag="hT",
                              bufs=1)
            wk1_res = persist.tile([P, DB, DB, P], BF16, name="wk1_res",
                                   tag="wk1r", bufs=1)
            ctxT = persist.tile([P, DB, CTX], BF16, name="ctxT", tag="ctxT")

            # ------- phase 1a: adaln1 + Q/K/V projections, chunked -------
            # hT is a 2-chunk ring: adaln of chunk ck writes slot ck%2 while
            # the K/V matmuls of ck-1 drain the other slot. wk1 is resident
            # (one 2MB DMA) so K matmuls never wait on just-in-time weights.
            x_tiles = dict(enumerate(xpre))
            wq_tiles = {}
            for ib in range(2):
                w_t = wk.tile([P, DB, P], BF16, name=f"wq1_{ib}", tag="wibt",
                              bufs=2)
                nc.sync.dma_start(w_t, d["a1_wqT"][ib])
                wq_tiles[ib] = w_t
            nc.sync.dma_start(wk1_res, d["a1_wkT"])
            def ctx_prep(cc, pool):
                c_t = wk.tile([P, D], BF16, name=f"ctxt_{cc}", tag="xg",
                              bufs=3)
                nc.sync.dma_start(c_t, d["ctx"][cc * P:(cc + 1) * P, :])
                for db in range(DB):
                    ps_t = pool.tile([P, P], BF16, name=f"ptc_{cc}_{db}",
                                     tag="tr", bufs=4)
                    nc.tensor.transpose(ps_t, c_t[:, db * P:(db + 1) * P],
                                        idt)
                    if db % 2 == 0:
                        nc.scalar.copy(
                            ctxT[:, db, cc * P:(cc + 1) * P], ps_t)
                    else:
                        nc.vector.tensor_copy(
                            ctxT[:, db, cc * P:(cc + 1) * P], ps_t)

            with tc.tile_pool(name="ps1a", bufs=1, space="PSUM") as ps1a:
                # chunk-level software pipeline: stats(ck) on DVE run under
                # the K/V matmuls of ck-1; normalize+transpose of ck follows
                # them on the PE with everything already computed.
                pend = {}

                def stats_ck(ck):
                    for t in range(4):
                        ti = ck * 4 + t
                        if ti in x_tiles:
                            x_t = x_tiles.pop(ti)
                        else:
                            x_t = wk.tile([P, D], BF16, name=f"x1_{ti}",
                                          tag="xg", bufs=3)
                            nc.sync.dma_start(
                                x_t, d["x_bf"][ti * P:(ti + 1) * P, :])
                        pend[ti] = (x_t, _adaln_stats(nc, pools, x_t,
                                                      f"a1_{ti}"))

                def trans_ck(ck):
                    sl = ck % 2
                    for t in range(4):
                        ti = ck * 4 + t
                        x_t, st = pend.pop(ti)
                        _adaln_trans(
                            nc, pools, x_t, st,
                            lambda db, t=t, sl=sl:
                                hT[:, db, sl, t * P:(t + 1) * P],
                            ps1a, f"a1_{ti}")

                stats_ck(0)
                trans_ck(0)
                for ck in range(S // OWN):
                    c0 = ck * OWN
                    sl = ck % 2
                    if ck == 0:
                        # Q proj (own rows) with per-partition bias evict
                        for ib in range(DB):
                            if ib in wq_tiles:
                                w_t = wq_tiles.pop(ib)
                            else:
                                w_t = wk.tile([P, DB, P], BF16,
                                              name=f"wq1_{ib}", tag="wibt",
                                              bufs=2)
                                nc.sync.dma_start(w_t, d["a1_wqT"][ib])
                            ps = ps1a.tile([P, OWN], F32, name=f"q1_{ib}",
                                           tag="mm", bufs=3)
                            for db in range(DB):
                                nc.tensor.matmul(ps, w_t[:, db, :],
                                                 hT[:, db, 0, :],
                                                 start=(db == 0),
                                                 stop=(db == DB - 1))
                            nc.vector.tensor_scalar(
                                Q1T[:, ib, :], ps, bq1_sb[:, ib:ib + 1], None,
                                op0=ALU.add)
                    if ck + 1 < S // OWN:
                        stats_ck(ck + 1)
                    for ib in range(DB):
                        ps = ps1a.tile([P, OWN], F32, name=f"k1_{ck}_{ib}",
                                       tag="mm", bufs=3)
                        for db in range(DB):
                            nc.tensor.matmul(ps, wk1_res[:, ib, db, :],
                                             hT[:, db, sl, :],
                                             start=(db == 0),
                                             stop=(db == DB - 1))
                        nc.scalar.copy(K1T[:, ib, c0:c0 + OWN], ps)
                    for half in range(2):
                        wv_t = []
                        for db in range(DB):
                            w_t = wk.tile([P, 512], BF16,
                                          name=f"wv1_{ck}_{half}_{db}",
                                          tag="wrhs", bufs=9)
                            nc.sync.dma_start(
                                w_t,
                                d["a1_wv"][db, :, half * 512:(half + 1) * 512])
                            wv_t.append(w_t)
                        for cc in range(4):
                            ps = ps1a.tile([P, 512], F32,
                                           name=f"v1_{ck}_{half}_{cc}",
                                           tag="mm", bufs=3)
                            for db in range(DB):
                                nc.tensor.matmul(
                                    ps,
                                    hT[:, db, sl, cc * P:(cc + 1) * P],
                                    wv_t[db], start=(db == 0),
                                    stop=(db == DB - 1))
                            nc.vector.tensor_copy(
                                V1[:, ck * 4 + cc,
                                   half * 512:(half + 1) * 512], ps)
                    if ck + 1 < S // OWN:
                        trans_ck(ck + 1)

            # ------- phase 1b: attn1 core + out-proj + residual -------
            wo1_sb = persist.tile([P, NPAIR, D], BF16, name="wo1_sb",
                                  tag="wo", bufs=1)
            nc.sync.dma_start(
                wo1_sb, d["a1_wo"].rearrange("hp p j -> p hp j"))
            nc.sync.dma_start(
                x1s, d["x_own"].rearrange("(rc p) j -> p rc j", p=P))
            with tc.tile_pool(name="ps1b", bufs=1, space="PSUM") as ps1b:

                def x1_write(rc, half, ps):
                    sl = x1s[:, rc, half * 512:(half + 1) * 512]
                    nc.vector.tensor_tensor(sl, ps, sl, op=ALU.add)

                _mha_core(nc, pools, K1T, V1, Q1T, S // P, ps1b,
                          wo1_sb, None, x1_write, "m1")

            # ------- phase 2a: ctx prep + adaln2 + Q2 -------
            wo2_sb = persist.tile([P, NPAIR, D], BF16, name="wo2_sb",
                                  tag="wo", bufs=1)
            bo2_row = persist.tile([1, D], BF16, name="bo2_row", tag="brow")
            with tc.tile_pool(name="ps2a", bufs=1, space="PSUM") as ps2a:
                for cc in range(CTX // P):
                    ctx_prep(cc, ps2a)
                nc.sync.dma_start(
                    wo2_sb, d["a2_wo"].rearrange("hp p j -> p hp j"))
                nc.sync.dma_start(bo2_row, d["bo2"])
                for ib in range(DB):
                    w_t = wk.tile([P, DB, P], BF16, name=f"wk2_{ib}",
                                  tag="wibt", bufs=2)
                    nc.sync.dma_start(w_t, d["a2_wkT"][ib])
                    ps = ps2a.tile([P, CTX], F32, name=f"k2_{ib}", tag="mm",
                                   bufs=3)
                    for db in range(DB):
                        nc.tensor.matmul(ps, w_t[:, db, :], ctxT[:, db, :],
                                         start=(db == 0), stop=(db == DB - 1))
                    nc.scalar.copy(K2T[:, ib, :], ps)
                for half in range(2):
                    wv_t = []
                    for db in range(DB):
                        w_t = wk.tile([P, 512], BF16, name=f"wv2_{half}_{db}",
                                      tag="wrhs", bufs=9)
                        nc.sync.dma_start(
                            w_t, d["a2_wv"][db, :, half * 512:(half + 1) * 512])
                        wv_t.append(w_t)
                    for cc in range(CTX // P):
                        ps = ps2a.tile([P, 512], F32, name=f"v2_{half}_{cc}",
                                       tag="mm", bufs=3)
                        for db in range(DB):
                            nc.tensor.matmul(
                                ps, ctxT[:, db, cc * P:(cc + 1) * P],
                                wv_t[db], start=(db == 0),
                                stop=(db == DB - 1))
                        nc.vector.tensor_copy(
                            V2[:, cc, half * 512:(half + 1) * 512], ps)

                h2T = persist.tile([P, DB, OWN], BF16, name="h2T", tag="hT",
                                   bufs=1)
                Q2T = persist.tile([P, NPAIR, OWN], BF16, name="Q2T", tag="qT",
                                   bufs=1)
                for t in range(4):
                    _adaln_tile(
                        nc, pools, x1s[:, t, :],
                        lambda db, t=t: h2T[:, db, t * P:(t + 1) * P],
                        ps2a, f"a2_{t}")
                for ib in range(DB):
                    w_t = wk.tile([P, DB, P], BF16, name=f"wq2_{ib}",
                                  tag="wibt", bufs=2)
                    nc.sync.dma_start(w_t, d["a2_wqT"][ib])
                    ps = ps2a.tile([P, OWN], F32, name=f"q2_{ib}", tag="mm",
                                   bufs=3)
                    for db in range(DB):
                        nc.tensor.matmul(ps, w_t[:, db, :], h2T[:, db, :],
                                         start=(db == 0), stop=(db == DB - 1))
                    nc.vector.tensor_scalar(
                        Q2T[:, ib, :], ps, bq2_sb[:, ib:ib + 1], None,
                        op0=ALU.add)

            # ------- phase 2b: attn2 core -------
            with tc.tile_pool(name="ps2b", bufs=1, space="PSUM") as ps2b:

                def x2_write(rc, half, ps):
                    # x2 = ps (incl. bo2) + x1 overwrites x1s in place
                    sl = x1s[:, rc, half * 512:(half + 1) * 512]
                    nc.vector.tensor_tensor(sl, ps, sl, op=ALU.add)

                _mha_core(nc, pools, K2T, V2, Q2T, CTX // P, ps2b,
                          wo2_sb, bo2_row, x2_write, "m2")

            # ---------------- phase 3: adaln3 + gated FFN ----------------
            b2_row = persist.tile([1, D], BF16, name="b2_row", tag="brow")
            nc.sync.dma_start(b2_row, d["b2"])
            b1a_sb = const.tile([P, 32], F32, name="b1a_sb")
            nc.sync.dma_start(b1a_sb, d["b1a"])
            b1g_sb = const.tile([P, 32], F32, name="b1g_sb")
            nc.sync.dma_start(b1g_sb, d["b1g"])
            h3T = persist.tile([P, DB, OWN], BF16, name="h3T", tag="hT",
                               bufs=1)
            gT = persist.tile([P, 32, OWN], BF16, name="gT", tag="K1T")
            with tc.tile_pool(name="ps3t", bufs=1, space="PSUM") as ps3t:
                for t in range(4):
                    _adaln_tile(
                        nc, pools, x1s[:, t, :],
                        lambda db, t=t: h3T[:, db, t * P:(t + 1) * P],
                        ps3t, f"a3_{t}")
            with tc.tile_pool(name="ps3", bufs=1, space="PSUM") as ps3:
                ffacc0 = ps3.tile([P, 4, 512], F32, name="ffacc0",
                                  tag="ffacc", bufs=1)
                ones_row = pools["ones_row"]
                for rc in range(4):
                    nc.tensor.matmul(ffacc0[:, rc, :], ones_row,
                                     b2_row[0:1, 0:512], start=True,
                                     stop=False)
                w2_handles = {}

                def gT_mms(i, acc, w2_tile):
                    for rc in range(4):
                        nc.tensor.matmul(acc[:, rc, :],
                                         gT[:, i, rc * P:(rc + 1) * P],
                                         w2_tile, start=False, stop=(i == 31))

                for i in range(32):
                    wa_t = wk.tile([P, DB, P], BF16, name=f"w1a_{i}",
                                   tag="w1t", bufs=3)
                    nc.sync.dma_start(wa_t, d["w1"][i])
                    wg_t = wk.tile([P, DB, P], BF16, name=f"w1g_{i}",
                                   tag="w1t", bufs=3)
                    nc.sync.dma_start(wg_t, d["w1"][32 + i])
                    ps_a = ps3.tile([P, OWN], F32, name=f"ua_{i}", tag="mma",
                                    bufs=2)
                    ps_g = ps3.tile([P, OWN], F32, name=f"ug_{i}", tag="mmg",
                                    bufs=2)
                    for db in range(DB):
                        nc.tensor.matmul(ps_a, wa_t[:, db, :], h3T[:, db, :],
                                         start=(db == 0), stop=(db == DB - 1))
                    for db in range(DB):
                        nc.tensor.matmul(ps_g, wg_t[:, db, :], h3T[:, db, :],
                                         start=(db == 0), stop=(db == DB - 1))
                    if i >= 1:
                        gT_mms(i - 1, ffacc0, w2_handles.pop(i - 1))
                    gl = wk.tile([P, OWN], BF16, name=f"gl_{i}", tag="gl",
                                 bufs=2)
                    nc.scalar.activation(gl, ps_g, AF.Gelu,
                                         bias=b1g_sb[:, i:i + 1])
                    nc.vector.scalar_tensor_tensor(gT[:, i, :], ps_a,
                                                   b1a_sb[:, i:i + 1], gl,
                                                   op0=ALU.add, op1=ALU.mult)
                    w2_t = wk.tile([P, 512], BF16, name=f"w2a_{i}", tag="w2t",
                                   bufs=3)
                    nc.sync.dma_start(w2_t, d["w2"][i, :, 0:512])
                    w2_handles[i] = w2_t
                w2c_handles = {}
                for kb in range(2):
                    w2_t = wk.tile([P, 512], BF16, name=f"w2c_{kb}",
                                   tag="w2t", bufs=3)
                    nc.sync.dma_start(w2_t, d["w2"][kb, :, 512:1024])
                    w2c_handles[kb] = w2_t
                gT_mms(31, ffacc0, w2_handles.pop(31))
                for rc in range(4):
                    xo = wk.tile([P, 512], F32, name=f"xo3a_{rc}", tag="xout",
                                 bufs=2)
                    nc.vector.tensor_tensor(xo, ffacc0[:, rc, :],
                                            x1s[:, rc, 0:512], op=ALU.add)
                    nc.sync.dma_start(out_d[rc * P:(rc + 1) * P, 0:512], xo)
                ffacc1 = [
                    ps3.tile([P, OWN], F32, name=f"ffacc1_{rc}",
                             tag=("mma" if rc < 2 else "mmg"), bufs=2)
                    for rc in range(4)]
                for rc in range(4):
                    nc.tensor.matmul(ffacc1[rc], ones_row,
                                     b2_row[0:1, 512:1024], start=True,
                                     stop=False)
                for kb in range(32):
                    if kb in w2c_handles:
                        w2_t = w2c_handles.pop(kb)
                    else:
                        w2_t = wk.tile([P, 512], BF16, name=f"w2c_{kb}",
                                       tag="w2t", bufs=3)
                        nc.sync.dma_start(w2_t, d["w2"][kb, :, 512:1024])
                    for rc in range(4):
                        nc.tensor.matmul(ffacc1[rc],
                                         gT[:, kb, rc * P:(rc + 1) * P],
                                         w2_t,
                                         start=False, stop=(kb == 31))
                for rc in range(4):
                    xo = wk.tile([P, 512], F32, name=f"xo3b_{rc}", tag="xout",
                                 bufs=2)
                    nc.vector.tensor_tensor(xo, ffacc1[rc],
                                            x1s[:, rc, 512:1024], op=ALU.add)
                    nc.sync.dma_start(out_d[rc * P:(rc + 1) * P, 512:1024], xo)

    nc.compile()
    return nc


# --------------------------------------------------------------------------
# host side
# --------------------------------------------------------------------------

def host_prep(inputs):
    bf = lambda a: np.ascontiguousarray(np.asarray(a).astype(NPBF16))
    f32 = lambda a: np.ascontiguousarray(np.asarray(a).astype(np.float32))

    def wib(w):  # [D, INNER] -> [ib, p, db, j]
        return np.ascontiguousarray(
            np.asarray(w).reshape(DB, P, DB, P).transpose(2, 1, 0, 3)
            .astype(NPBF16))

    x = np.asarray(inputs["x"])
    t = np.asarray(inputs["t"])
    context = np.asarray(inputs["context"])

    # AdaLN emb on host; fold scale into weight rows, shift into biases.
    per_batch = []
    for b in range(B):
        e = {}
        for i in (1, 2, 3):
            v = (t[b, 0].astype(np.float64)
                 @ np.asarray(inputs[f"norm{i}_w"]).astype(np.float64)
                 + np.asarray(inputs[f"norm{i}_b"]).astype(np.float64))
            e[i] = (1.0 + v[:D], v[D:])          # (1+scale, shift)
        m = {}
        wq1 = np.asarray(inputs["attn1_wq"]).astype(np.float64)
        wk1 = np.asarray(inputs["attn1_wk"]).astype(np.float64)
        wv1 = np.asarray(inputs["attn1_wv"]).astype(np.float64)
        wo1 = np.asarray(inputs["attn1_wo"]).astype(np.float64)
        s1, h1 = e[1]
        m["a1_wqT"] = wib(wq1 * s1[:, None])
        m["a1_wkT"] = np.ascontiguousarray(
            wib(wk1 * s1[:, None]).transpose(1, 0, 2, 3))
        m["a1_wv"] = bf((wv1 * s1[:, None]).reshape(DB, P, INNER))
        m["a1_wo"] = bf(wo1.reshape(NPAIR, P, D))
        m["bq1"] = f32((h1 @ wq1).reshape(DB, P).T)
        bo1 = (np.asarray(inputs["attn1_bo"]).astype(np.float64)
               + (h1 @ wv1) @ wo1)                # V-shift folded through wo
        wq2 = np.asarray(inputs["attn2_wq"]).astype(np.float64)
        s2, h2 = e[2]
        m["a2_wqT"] = wib(wq2 * s2[:, None])
        m["a2_wkT"] = wib(np.asarray(inputs["attn2_wk"]))
        m["a2_wv"] = bf(np.asarray(inputs["attn2_wv"]).reshape(DB, P, INNER))
        m["a2_wo"] = bf(np.asarray(inputs["attn2_wo"]).reshape(NPAIR, P, D))
        m["bq2"] = f32((h2 @ wq2).reshape(DB, P).T)
        m["bo2"] = bf(np.asarray(inputs["attn2_bo"]).reshape(1, D))
        w1 = np.asarray(inputs["ff_w1"]).astype(np.float64)
        s3, h3 = e[3]
        m["w1"] = np.ascontiguousarray(
            (w1 * s3[:, None]).reshape(DB, P, 64, P)
            .transpose(2, 1, 0, 3).astype(NPBF16))
        b1 = np.asarray(inputs["ff_b1"]).astype(np.float64) + h3 @ w1
        m["b1a"] = f32(b1[:DFF].reshape(32, P).T)
        m["b1g"] = f32(b1[DFF:].reshape(32, P).T)
        m["w2"] = bf(np.asarray(inputs["ff_w2"]).reshape(32, P, D))
        m["b2"] = bf(np.asarray(inputs["ff_b2"]).reshape(1, D))
        m["ctx"] = bf(context[b])
        m["_bo1"] = bo1
        per_batch.append(m)

    in_maps = []
    for c in range(NCORES):
        b, q = c // 4, c % 4
        pb = per_batch[b]
        m = {k: v for k, v in pb.items() if not k.startswith("_")}
        m["x_bf"] = bf(np.roll(x[b], -q * OWN, axis=0))
        m["x_own"] = f32(x[b, q * OWN:(q + 1) * OWN].astype(np.float64)
                         + pb["_bo1"][None, :])
        in_maps.append(m)
    return in_maps


_CACHE = {}


def kernel(**inputs):
    if "nc" not in _CACHE:
        _CACHE["nc"] = build_program()
    nc = _CACHE["nc"]
    key = tuple(id(inputs[k]) for k in sorted(inputs))
    if _CACHE.get("prep_key") != key:
        _CACHE["in_maps"] = host_prep(inputs)
        _CACHE["prep_key"] = key
        _CACHE["prep_refs"] = inputs
    in_maps = _CACHE["in_maps"]
    res = bass_utils.run_bass_kernel_spmd(
        nc, in_maps, core_ids=list(range(NCORES)), trace=False)
    _CACHE["last_exec_ns"] = res.exec_time_ns
    _CACHE["last_results"] = res
    out = np.empty((B, S, D), np.float32)
    for c in range(NCORES):
        b, q = c // 4, c % 4
        out[b, q * OWN:(q + 1) * OWN] = res.results[c]["out"]
    return out


# revision 29
# speedup vs baseline: 1.1674x; 1.1674x over previous
"""BasicTransformerBlock Trainium2 kernel, v3.

Sharding: 8 cores = 2 batch groups x 4 sequence shards. The host rotates each
core's rows so its own 512 rows are always rows 0..511 (pure SPMD, no
collectives). Attention is key-order invariant, so each core recomputes
adaLN1 + K/V projections over the full (rotated) 2048-row sequence of its
batch; Q/attention/out-proj/FFN are local to its own 512 rows.

Key differences vs v2:
- AdaLN scale/shift folded into the weights on HOST:
    adaln(x) @ W = LN(x) @ (diag(1+s) W) + shift @ W
  so the device only computes plain LN (stats + per-partition normalize),
  transpose, and a raw copy-evict. K-projection bias is dropped entirely
  (additive per-query constants cancel in softmax); V bias folds through
  the out-projection into bo; attn1's bo folds into the residual rows
  shipped from host; Q bias is a per-partition scalar in the QT evict.
- Softmax tail rebuilt: denominators accumulate into one [1,2,512] PSUM
  row pair, eviction is one small DVE copy, partition-broadcast is a K=1
  rank-1 PE matmul, reciprocal is the fast custom-DVE approx on all 128
  partitions. No more 4us single-lane reciprocals / gpsimd broadcast, so
  the PE never idles long enough for HAM to re-throttle.
- attn2/FFN biases (bo2, b2) enter the PSUM accumulation as K=1 ones-row
  matmuls instead of extra DVE passes.
- FFN gT@W2 matmuls lag one iteration behind the W1 matmuls so the
  in-order PE queue never head-of-line blocks on the gelu/GLU chain.
- Phase 1a is chunk-interleaved: adaLN of 512 rows, then K/V projections
  of those rows, keeping the PE dense from the start.

Heavy matmuls in bf16 with fp32 PSUM accumulation.
"""

import numpy as np
import ml_dtypes

import concourse.bass as bass
import concourse.bacc as bacc
import concourse.mybir as mybir
import concourse.tile as tile
from concourse import bass_utils
from concourse.masks import make_identity

P = 128
B, S, CTX, D, H, DH = 2, 2048, 256, 1024, 16, 64
INNER = H * DH          # 1024
DFF = 4 * D             # 4096
NCORES = 8
OWN = 512               # rows owned per core
NPAIR = H // 2          # 8 head pairs
DB = D // P             # 8 model-dim blocks
F32 = mybir.dt.float32
BF16 = mybir.dt.bfloat16
NPBF16 = ml_dtypes.bfloat16

AF = mybir.ActivationFunctionType
ALU = mybir.AluOpType


def _adaln_stats(nc, pools, x_src, name):
    """LN stats for one 128-row tile: DVE/ScalarE only, no PE. Returns the
    (nmr, rstd) per-partition scalars for the later normalize pass."""
    wk = pools["wk"]
    stats = wk.tile([P, 2, 6], F32, name=f"st_{name}", tag="stats", bufs=5)
    nc.vector.bn_stats(stats[:, 0, :], x_src[:, 0:512])
    nc.vector.bn_stats(stats[:, 1, :], x_src[:, 512:1024])
    mv = wk.tile([P, 2], F32, name=f"mv_{name}", tag="mv", bufs=5)
    nc.vector.bn_aggr(mv, stats)
    sd = wk.tile([P, 1], F32, name=f"sd_{name}", tag="sd", bufs=5)
    nc.scalar.activation(sd, mv[:, 1:2], AF.Sqrt, bias=pools["eps"][:, 0:1])
    rstd = wk.tile([P, 1], F32, name=f"rs_{name}", tag="rstd", bufs=5)
    nc.vector.reciprocal(rstd, sd)
    nmr = wk.tile([P, 1], F32, name=f"nm_{name}", tag="nmr", bufs=5)
    nc.vector.tensor_scalar(nmr, mv[:, 0:1], rstd, -1.0,
                            op0=ALU.mult, op1=ALU.mult)
    return nmr, rstd


def _adaln_trans(nc, pools, x_src, st, hT_dst, tr_pool, name):
    """Normalize (per-partition affine on ScalarE) + transpose + evict."""
    wk = pools["wk"]
    nmr, rstd = st
    xn = wk.tile([P, D], BF16, name=f"xn_{name}", tag="xn", bufs=2)
    nc.scalar.activation(xn, x_src, AF.Identity, bias=nmr[:, 0:1],
                         scale=rstd[:, 0:1])
    for db in range(DB):
        ps_t = tr_pool.tile([P, P], BF16, name=f"pt_{name}_{db}", tag="tr",
                            bufs=4)
        nc.tensor.transpose(ps_t, xn[:, db * P:(db + 1) * P], pools["idt"])
        if db % 2 == 0:
            nc.scalar.copy(hT_dst(db), ps_t)
        else:
            nc.vector.tensor_copy(hT_dst(db), ps_t)


def _adaln_tile(nc, pools, x_src, hT_dst, tr_pool, name):
    st = _adaln_stats(nc, pools, x_src, name)
    _adaln_trans(nc, pools, x_src, st, hT_dst, tr_pool, name)


def _mha_core(nc, pools, KT, VT, QT, n_kb, ps_pool, wo_sb, bias_row,
              x_dst_write, name):
    """Attention core + out-projection (+ optional bias matmul) + residual.

    The kb loop is software-pipelined: scores/exp for step kb issue before
    the PV/denominator matmuls of step kb-1. Softmax normalization:
    denominators for both heads accumulate into one [1,2,512] PSUM row,
    get evicted bf16, rank-1 broadcast across partitions on the PE, and
    fast-approx reciprocated on the DVE over all 128 lanes.

    KT: [128, 8, n_kb*128] bf16 (pair-dim on partitions, keys on free)
    VT: [128, n_kb, 1024] bf16  (key rows on partitions, inner on free)
    QT: [128, 8, 512] bf16
    wo_sb: [128, 8, 1024] bf16 resident out-proj weights
    bias_row: [1, D] bf16 SBUF row added via K=1 matmul, or None
    x_dst_write(rc, half, ps_ap): consume finished out-proj PSUM chunk
    """
    wk = pools["wk"]
    outT = pools["outT"]
    ones = pools["ones"]          # [P, 1] bf16: K=128, M=1 (denominator)
    ones_row = pools["ones_row"]  # [1, P] bf16: K=1, M=128 (broadcast)

    for hp in range(NPAIR):
        pv = ps_pool.tile([P, 512], F32, name=f"pv_{name}_{hp}", tag="pv",
                          bufs=2)
        dn = ps_pool.tile([P, 2, 512], F32, name=f"dn_{name}_{hp}", tag="dn",
                          bufs=1)
        probs_q = {}

        def scores_step(kb):
            ps_s = ps_pool.tile([P, 2, 512], F32, name=f"s_{name}_{hp}_{kb}",
                                tag="sc", bufs=2)
            nc.tensor.matmul(ps_s[:, 0, :], KT[0:64, hp, kb * P:(kb + 1) * P],
                             QT[0:64, hp, :], start=True, stop=True)
            nc.tensor.matmul(ps_s[:, 1, :], KT[64:128, hp, kb * P:(kb + 1) * P],
                             QT[64:128, hp, :], start=True, stop=True,
                             tile_position=(64, 0))
            probs = wk.tile([P, 2, 512], BF16, name=f"pr_{name}_{hp}_{kb}",
                            tag="probs", bufs=2)
            nc.scalar.activation(probs, ps_s, AF.Exp, scale=0.125)
            probs_q[kb] = probs

        def pv_step(kb):
            probs = probs_q.pop(kb)
            nc.tensor.matmul(pv[0:64, :], VT[:, kb, hp * P:hp * P + 64],
                             probs[:, 0, :], start=(kb == 0),
                             stop=(kb == n_kb - 1))
            nc.tensor.matmul(pv[64:128, :],
                             VT[:, kb, hp * P + 64:hp * P + 128],
                             probs[:, 1, :], start=(kb == 0),
                             stop=(kb == n_kb - 1), tile_position=(0, 64))
            nc.tensor.matmul(dn[0:1, 0, :], ones, probs[:, 0, :],
                             start=(kb == 0), stop=(kb == n_kb - 1))
            nc.tensor.matmul(dn[0:1, 1, :], ones, probs[:, 1, :],
                             start=(kb == 0), stop=(kb == n_kb - 1))

        for kb in range(n_kb):
            scores_step(kb)
            if kb >= 1:
                pv_step(kb - 1)
        pv_step(n_kb - 1)

        dn_sb = wk.tile([1, 2, 512], BF16, name=f"dsb_{name}_{hp}",
                        tag="dnsb", bufs=1)
        nc.vector.tensor_copy(dn_sb, dn[0:1, :, :])
        bc = ps_pool.tile([P, 512], F32, name=f"bc_{name}_{hp}", tag="sc",
                          bufs=2)
        nc.tensor.matmul(bc[0:64, :], ones_row[0:1, 0:64], dn_sb[0:1, 0, :],
                         start=True, stop=True)
        nc.tensor.matmul(bc[64:128, :], ones_row[0:1, 0:64],
                         dn_sb[0:1, 1, :], start=True, stop=True,
                         tile_position=(0, 64))
        rec = wk.tile([P, 512], F32, name=f"rec_{name}_{hp}", tag="rec",
                      bufs=1)
        nc.vector.reciprocal_approx_fast(rec, bc)
        nc.vector.tensor_tensor(outT[0:64, hp, :], pv[0:64, :],
                                rec[0:64, :], op=ALU.mult)
        nc.vector.tensor_tensor(outT[64:128, hp, :], pv[64:128, :],
                                rec[64:128, :], op=ALU.mult)

    # out-projection (+bias via K=1 matmul) + residual, row-chunk outer so
    # the residual stream completes chunk-by-chunk.
    for rc in range(4):
        for half in range(2):
            ps = ps_pool.tile([P, 512], F32, name=f"op_{name}_{rc}_{half}",
                              tag="sc", bufs=2)
            if bias_row is not None:
                nc.tensor.matmul(ps, ones_row,
                                 bias_row[0:1, half * 512:(half + 1) * 512],
                                 start=True, stop=False)
            for hp in range(NPAIR):
                nc.tensor.matmul(ps, outT[:, hp, rc * P:(rc + 1) * P],
                                 wo_sb[:, hp, half * 512:(half + 1) * 512],
                                 start=(hp == 0 and bias_row is None),
                                 stop=(hp == NPAIR - 1))
            x_dst_write(rc, half, ps)


def build_program():
    nc = bacc.Bacc("TRN2", target_bir_lowering=False, debug=False,
                   num_devices=NCORES)
    d = {}

    def din(nm, shape, dt):
        d[nm] = nc.dram_tensor(nm, shape, dt, kind="ExternalInput").ap()
        return d[nm]

    din("x_bf", [S, D], BF16)          # full rotated rows, LN input
    din("x_own", [OWN, D], F32)        # own rows + bo1 folded, residual
    din("ctx", [CTX, D], BF16)
    din("bq1", [P, DB], F32)           # Q1 bias, feature-major columns
    din("bq2", [P, DB], F32)
    for a in ("a1", "a2"):
        din(f"{a}_wqT", [DB, P, DB, P], BF16)   # [ib, p, db, j]
        din(f"{a}_wkT", [DB, P, DB, P] if a == "a2" else [P, DB, DB, P],
            BF16)
        din(f"{a}_wv", [DB, P, INNER], BF16)    # [db, p, j]
        din(f"{a}_wo", [NPAIR, P, D], BF16)     # [hp, p, j]
    din("bo2", [1, D], BF16)
    din("w1", [64, P, DB, P], BF16)             # [chunk, p, db, j]
    din("b1a", [P, 32], F32)
    din("b1g", [P, 32], F32)
    din("w2", [32, P, D], BF16)                 # [kb, p, j]
    din("b2", [1, D], BF16)
    out_d = nc.dram_tensor("out", [OWN, D], F32, kind="ExternalOutput").ap()

    with tile.TileContext(nc) as tc:
        import contextlib
        with contextlib.ExitStack() as ctx:
            const = ctx.enter_context(tc.tile_pool(name="const", bufs=1))
            persist = ctx.enter_context(tc.tile_pool(name="persist", bufs=1))
            wk = ctx.enter_context(tc.tile_pool(name="wkp", bufs=1))

            pools = {"wk": wk}
            xpre = []
            for ti in range(3):
                x_t = wk.tile([P, D], BF16, name=f"x1_{ti}", tag="xg", bufs=3)
                nc.sync.dma_start(x_t, d["x_bf"][ti * P:(ti + 1) * P, :])
                xpre.append(x_t)
            idt = const.tile([P, P], BF16, name="idt")
            make_identity(nc, idt)
            pools["idt"] = idt
            ones_bf = const.tile([P, 1], BF16, name="ones_bf")
            nc.vector.memset(ones_bf, 1.0)
            pools["ones"] = ones_bf
            ones_row = const.tile([1, P], BF16, name="ones_row")
            nc.vector.memset(ones_row, 1.0)
            pools["ones_row"] = ones_row
            eps_t = const.tile([P, 1], F32, name="eps_t")
            nc.vector.memset(eps_t, 1e-5)
            pools["eps"] = eps_t
            bq1_sb = const.tile([P, DB], F32, name="bq1_sb")
            nc.sync.dma_start(bq1_sb, d["bq1"])
            bq2_sb = const.tile([P, DB], F32, name="bq2_sb")
            nc.sync.dma_start(bq2_sb, d["bq2"])

            K1T = persist.tile([P, NPAIR, S], BF16, name="K1T", tag="K1T")
            V1 = persist.tile([P, S // P, INNER], BF16, name="V1", tag="V1")
            Q1T = persist.tile([P, NPAIR, OWN], BF16, name="Q1T", tag="qT",
                               bufs=1)
            outT = persist.tile([P, NPAIR, OWN], BF16, name="outT", tag="outT")
            pools["outT"] = outT
            K2T = persist.tile([P, NPAIR, CTX], BF16, name="K2T", tag="K2T")
            V2 = persist.tile([P, CTX // P, INNER], BF16, name="V2", tag="V2")
            x1s = persist.tile([P, 4, D], F32, name="x1s", tag="x1s")
            hT = persist.tile([P, DB, 2, OWN], BF16, name="hT", tag="hT",
                              bufs=1)
            wk1_res = persist.tile([P, DB, DB, P], BF16, name="wk1_res",
                                   tag="wk1r", bufs=1)
            ctxT = persist.tile([P, DB, CTX], BF16, name="ctxT", tag="ctxT")

            # ------- phase 1a: adaln1 + Q/K/V projections, chunked -------
            # hT is a 2-chunk ring: adaln of chunk ck writes slot ck%2 while
            # the K/V matmuls of ck-1 drain the other slot. wk1 is resident
            # (one 2MB DMA) so K matmuls never wait on just-in-time weights.
            x_tiles = dict(enumerate(xpre))
            wq_tiles = {}
            for ib in range(2):
                w_t = wk.tile([P, DB, P], BF16, name=f"wq1_{ib}", tag="wibt",
                              bufs=2)
                nc.sync.dma_start(w_t, d["a1_wqT"][ib])
                wq_tiles[ib] = w_t
            nc.sync.dma_start(wk1_res, d["a1_wkT"])
            def ctx_prep(cc, pool):
                c_t = wk.tile([P, D], BF16, name=f"ctxt_{cc}", tag="xg",
                              bufs=3)
                nc.sync.dma_start(c_t, d["ctx"][cc * P:(cc + 1) * P, :])
                for db in range(DB):
                    ps_t = pool.tile([P, P], BF16, name=f"ptc_{cc}_{db}",
                                     tag="tr", bufs=4)
                    nc.tensor.transpose(ps_t, c_t[:, db * P:(db + 1) * P],
                                        idt)
                    if db % 2 == 0:
                        nc.scalar.copy(
                            ctxT[:, db, cc * P:(cc + 1) * P], ps_t)
                    else:
                        nc.vector.tensor_copy(
                            ctxT[:, db, cc * P:(cc + 1) * P], ps_t)

            with tc.tile_pool(name="ps1a", bufs=1, space="PSUM") as ps1a:
                # chunk-level software pipeline: stats(ck) on DVE run under
                # the K/V matmuls of ck-1; normalize+transpose of ck follows
                # them on the PE with everything already computed.
                pend = {}

                def stats_ck(ck):
                    for t in range(4):
                        ti = ck * 4 + t
                        if ti in x_tiles:
                            x_t = x_tiles.pop(ti)
                        else:
                            x_t = wk.tile([P, D], BF16, name=f"x1_{ti}",
                                          tag="xg", bufs=3)
                            nc.sync.dma_start(
                                x_t, d["x_bf"][ti * P:(ti + 1) * P, :])
                        pend[ti] = (x_t, _adaln_stats(nc, pools, x_t,
                                                      f"a1_{ti}"))

                def trans_ck(ck):
                    sl = ck % 2
                    for t in range(4):
                        ti = ck * 4 + t
                        x_t, st = pend.pop(ti)
                        _adaln_trans(
                            nc, pools, x_t, st,
                            lambda db, t=t, sl=sl:
                                hT[:, db, sl, t * P:(t + 1) * P],
                            ps1a, f"a1_{ti}")

                stats_ck(0)
                trans_ck(0)
                for ck in range(S // OWN):
                    c0 = ck * OWN
                    sl = ck % 2
                    if ck == 0:
                        # Q proj (own rows) with per-partition bias evict
                        for ib in range(DB):
                            if ib in wq_tiles:
                                w_t = wq_tiles.pop(ib)
                            else:
                                w_t = wk.tile([P, DB, P], BF16,
                                              name=f"wq1_{ib}", tag="wibt",
                                              bufs=2)
                                nc.sync.dma_start(w_t, d["a1_wqT"][ib])
                            ps = ps1a.tile([P, OWN], F32, name=f"q1_{ib}",
                                           tag="mm", bufs=3)
                            for db in range(DB):
                                nc.tensor.matmul(ps, w_t[:, db, :],
                                                 hT[:, db, 0, :],
                                                 start=(db == 0),
                                                 stop=(db == DB - 1))
                            nc.vector.tensor_scalar(
                                Q1T[:, ib, :], ps, bq1_sb[:, ib:ib + 1], None,
                                op0=ALU.add)
                    if ck + 1 < S // OWN:
                        stats_ck(ck + 1)
                    for ib in range(DB):
                        ps = ps1a.tile([P, OWN], F32, name=f"k1_{ck}_{ib}",
                                       tag="mm", bufs=3)
                        for db in range(DB):
                            nc.tensor.matmul(ps, wk1_res[:, ib, db, :],
                                             hT[:, db, sl, :],
                                             start=(db == 0),
                                             stop=(db == DB - 1))
                        nc.scalar.copy(K1T[:, ib, c0:c0 + OWN], ps)
                    for half in range(2):
                        wv_t = []
                        for db in range(DB):
                            w_t = wk.tile([P, 512], BF16,
                                          name=f"wv1_{ck}_{half}_{db}",
                                          tag="wrhs", bufs=9)
                            nc.sync.dma_start(
                                w_t,
                                d["a1_wv"][db, :, half * 512:(half + 1) * 512])
                            wv_t.append(w_t)
                        for cc in range(4):
                            ps = ps1a.tile([P, 512], F32,
                                           name=f"v1_{ck}_{half}_{cc}",
                                           tag="mm", bufs=3)
                            for db in range(DB):
                                nc.tensor.matmul(
                                    ps,
                                    hT[:, db, sl, cc * P:(cc + 1) * P],
                                    wv_t[db], start=(db == 0),
                                    stop=(db == DB - 1))
                            nc.vector.tensor_copy(
                                V1[:, ck * 4 + cc,
                                   half * 512:(half + 1) * 512], ps)
                    if ck + 1 < S // OWN:
                        trans_ck(ck + 1)

            # ------- phase 1b: attn1 core + out-proj + residual -------
            wo1_sb = persist.tile([P, NPAIR, D], BF16, name="wo1_sb",
                                  tag="wo", bufs=1)
            nc.sync.dma_start(
                wo1_sb, d["a1_wo"].rearrange("hp p j -> p hp j"))
            nc.sync.dma_start(
                x1s, d["x_own"].rearrange("(rc p) j -> p rc j", p=P))
            with tc.tile_pool(name="ps1b", bufs=1, space="PSUM") as ps1b:

                def x1_write(rc, half, ps):
                    sl = x1s[:, rc, half * 512:(half + 1) * 512]
                    nc.vector.tensor_tensor(sl, ps, sl, op=ALU.add)

                _mha_core(nc, pools, K1T, V1, Q1T, S // P, ps1b,
                          wo1_sb, None, x1_write, "m1")

            # ------- phase 2a: ctx prep + adaln2 + Q2 -------
            wo2_sb = persist.tile([P, NPAIR, D], BF16, name="wo2_sb",
                                  tag="wo", bufs=1)
            bo2_row = persist.tile([1, D], BF16, name="bo2_row", tag="brow")
            with tc.tile_pool(name="ps2a", bufs=1, space="PSUM") as ps2a:
                for cc in range(CTX // P):
                    ctx_prep(cc, ps2a)
                nc.sync.dma_start(
                    wo2_sb, d["a2_wo"].rearrange("hp p j -> p hp j"))
                nc.sync.dma_start(bo2_row, d["bo2"])
                for ib in range(DB):
                    w_t = wk.tile([P, DB, P], BF16, name=f"wk2_{ib}",
                                  tag="wibt", bufs=2)
                    nc.sync.dma_start(w_t, d["a2_wkT"][ib])
                    ps = ps2a.tile([P, CTX], F32, name=f"k2_{ib}", tag="mm",
                                   bufs=3)
                    for db in range(DB):
                        nc.tensor.matmul(ps, w_t[:, db, :], ctxT[:, db, :],
                                         start=(db == 0), stop=(db == DB - 1))
                    nc.scalar.copy(K2T[:, ib, :], ps)
                for half in range(2):
                    wv_t = []
                    for db in range(DB):
                        w_t = wk.tile([P, 512], BF16, name=f"wv2_{half}_{db}",
                                      tag="wrhs", bufs=9)
                        nc.sync.dma_start(
                            w_t, d["a2_wv"][db, :, half * 512:(half + 1) * 512])
                        wv_t.append(w_t)
                    for cc in range(CTX // P):
                        ps = ps2a.tile([P, 512], F32, name=f"v2_{half}_{cc}",
                                       tag="mm", bufs=3)
                        for db in range(DB):
                            nc.tensor.matmul(
                                ps, ctxT[:, db, cc * P:(cc + 1) * P],
                                wv_t[db], start=(db == 0),
                                stop=(db == DB - 1))
                        nc.vector.tensor_copy(
                            V2[:, cc, half * 512:(half + 1) * 512], ps)

                h2T = persist.tile([P, DB, OWN], BF16, name="h2T", tag="hT",
                                   bufs=1)
                Q2T = persist.tile([P, NPAIR, OWN], BF16, name="Q2T", tag="qT",
                                   bufs=1)
                for t in range(4):
                    _adaln_tile(
                        nc, pools, x1s[:, t, :],
                        lambda db, t=t: h2T[:, db, t * P:(t + 1) * P],
                        ps2a, f"a2_{t}")
                for ib in range(DB):
                    w_t = wk.tile([P, DB, P], BF16, name=f"wq2_{ib}",
                                  tag="wibt", bufs=2)
                    nc.sync.dma_start(w_t, d["a2_wqT"][ib])
                    ps = ps2a.tile([P, OWN], F32, name=f"q2_{ib}", tag="mm",
                                   bufs=3)
                    for db in range(DB):
                        nc.tensor.matmul(ps, w_t[:, db, :], h2T[:, db, :],
                                         start=(db == 0), stop=(db == DB - 1))
                    nc.vector.tensor_scalar(
                        Q2T[:, ib, :], ps, bq2_sb[:, ib:ib + 1], None,
                        op0=ALU.add)

            # ------- phase 2b: attn2 core -------
            with tc.tile_pool(name="ps2b", bufs=1, space="PSUM") as ps2b:

                def x2_write(rc, half, ps):
                    # x2 = ps (incl. bo2) + x1 overwrites x1s in place
                    sl = x1s[:, rc, half * 512:(half + 1) * 512]
                    nc.vector.tensor_tensor(sl, ps, sl, op=ALU.add)

                _mha_core(nc, pools, K2T, V2, Q2T, CTX // P, ps2b,
                          wo2_sb, bo2_row, x2_write, "m2")

            # ---------------- phase 3: adaln3 + gated FFN ----------------
            b2_row = persist.tile([1, D], BF16, name="b2_row", tag="brow")
            nc.sync.dma_start(b2_row, d["b2"])
            b1a_sb = const.tile([P, 32], F32, name="b1a_sb")
            nc.sync.dma_start(b1a_sb, d["b1a"])
            b1g_sb = const.tile([P, 32], F32, name="b1g_sb")
            nc.sync.dma_start(b1g_sb, d["b1g"])
            h3T = persist.tile([P, DB, OWN], BF16, name="h3T", tag="hT",
                               bufs=1)
            gT = persist.tile([P, 32, OWN], BF16, name="gT", tag="K1T")
            with tc.tile_pool(name="ps3t", bufs=1, space="PSUM") as ps3t:
                for t in range(4):
                    _adaln_tile(
                        nc, pools, x1s[:, t, :],
                        lambda db, t=t: h3T[:, db, t * P:(t + 1) * P],
                        ps3t, f"a3_{t}")
            with tc.tile_pool(name="ps3", bufs=1, space="PSUM") as ps3:
                ffacc0 = ps3.tile([P, 4, 512], F32, name="ffacc0",
                                  tag="ffacc", bufs=1)
                ones_row = pools["ones_row"]
                for rc in range(4):
                    nc.tensor.matmul(ffacc0[:, rc, :], ones_row,
                                     b2_row[0:1, 0:512], start=True,
                                     stop=False)
                w2_handles = {}

                def gT_mms(i, acc, w2_tile):
                    for rc in range(4):
                        nc.tensor.matmul(acc[:, rc, :],
                                         gT[:, i, rc * P:(rc + 1) * P],
                                         w2_tile, start=False, stop=(i == 31))

                for i in range(32):
                    wa_t = wk.tile([P, DB, P], BF16, name=f"w1a_{i}",
                                   tag="w1t", bufs=3)
                    nc.sync.dma_start(wa_t, d["w1"][i])
                    wg_t = wk.tile([P, DB, P], BF16, name=f"w1g_{i}",
                                   tag="w1t", bufs=3)
                    nc.sync.dma_start(wg_t, d["w1"][32 + i])
                    ps_a = ps3.tile([P, OWN], F32, name=f"ua_{i}", tag="mma",
                                    bufs=2)
                    ps_g = ps3.tile([P, OWN], F32, name=f"ug_{i}", tag="mmg",
                                    bufs=2)
                    for db in range(DB):
                        nc.tensor.matmul(ps_a, wa_t[:, db, :], h3T[:, db, :],
                                         start=(db == 0), stop=(db == DB - 1))
                    for db in range(DB):
                        nc.tensor.matmul(ps_g, wg_t[:, db, :], h3T[:, db, :],
                                         start=(db == 0), stop=(db == DB - 1))
                    if i >= 1:
                        gT_mms(i - 1, ffacc0, w2_handles.pop(i - 1))
                    gl = wk.tile([P, OWN], BF16, name=f"gl_{i}", tag="gl",
                                 bufs=2)
                    nc.scalar.activation(gl, ps_g, AF.Gelu,
                                         bias=b1g_sb[:, i:i + 1])
                    nc.vector.scalar_tensor_tensor(gT[:, i, :], ps_a,
                                                   b1a_sb[:, i:i + 1], gl,
                                                   op0=ALU.add, op1=ALU.mult)
                    w2_t = wk.tile([P, 512], BF16, name=f"w2a_{i}", tag="w2t",
                                   bufs=3)
                    nc.sync.dma_start(w2_t, d["w2"][i, :, 0:512])
                    w2_handles[i] = w2_t
                w2c_handles = {}
                for kb in range(2):
                    w2_t = wk.tile([P, 512], BF16, name=f"w2c_{kb}",
                                   tag="w2t", bufs=3)
                    nc.sync.dma_start(w2_t, d["w2"][kb, :, 512:1024])
                    w2c_handles[kb] = w2_t
                gT_mms(31, ffacc0, w2_handles.pop(31))
                for rc in range(4):
                    xo = wk.tile([P, 512], F32, name=f"xo3a_{rc}", tag="xout",
                                 bufs=2)
                    nc.vector.tensor_tensor(xo, ffacc0[:, rc, :],
                                            x1s[:, rc, 0:512], op=ALU.add)
                    nc.sync.dma_start(out_d[rc * P:(rc + 1) * P, 0:512], xo)
                ffacc1 = [
                    ps3.tile([P, OWN], F32, name=f"ffacc1_{rc}",
                             tag=("mma" if rc < 2 else "mmg"), bufs=2)
                    for rc in range(4)]
                for rc in range(4):
                    nc.tensor.matmul(ffacc1[rc], ones_row,
                                     b2_row[0:1, 512:1024], start=True,
                                     stop=False)
                for kb in range(32):
                    if kb in w2c_handles:
                        w2_t = w2c_handles.pop(kb)
                    else:
                        w2_t = wk.tile([P, 512], BF16, name=f"w2c_{kb}",
                                       tag="w2t", bufs=3)
                        nc.sync.dma_start(w2_t, d["w2"][kb, :, 512:1024])
                    for rc in range(4):
                        nc.tensor.matmul(ffacc1[rc],
                                         gT[:, kb, rc * P:(rc + 1) * P],
                                         w2_t,
                                         start=False, stop=(kb == 31))
                for rc in range(4):
                    xo = wk.tile([P, 512], F32, name=f"xo3b_{rc}", tag="xout",
                                 bufs=2)
                    nc.vector.tensor_tensor(xo, ffacc1[rc],
                                            x1s[:, rc, 512:1024], op=ALU.add)
                    nc.sync.dma_start(out_d[rc * P:(rc + 1) * P, 512:1024], xo)

    nc.compile()
    return nc


# --------------------------------------------------------------------------
# host side
# --------------------------------------------------------------------------

def host_prep(inputs):
    bf = lambda a: np.ascontiguousarray(np.asarray(a).astype(NPBF16))
    f32 = lambda a: np.ascontiguousarray(np.asarray(a).astype(np.float32))

    def wib(w):  # [D, INNER] -> [ib, p, db, j]
        return np.ascontiguousarray(
            np.asarray(w).reshape(DB, P, DB, P).transpose(2, 1, 0, 3)
            .astype(NPBF16))

    x = np.asarray(inputs["x"])
    t = np.asarray(inputs["t"])
    context = np.asarray(inputs["context"])

    # AdaLN emb on host; fold scale into weight rows, shift into biases.
    per_batch = []
    for b in range(B):
        e = {}
        for i in (1, 2, 3):
            v = (t[b, 0].astype(np.float64)
                 @ np.asarray(inputs[f"norm{i}_w"]).astype(np.float64)
                 + np.asarray(inputs[f"norm{i}_b"]).astype(np.float64))
            e[i] = (1.0 + v[:D], v[D:])          # (1+scale, shift)
        m = {}
        wq1 = np.asarray(inputs["attn1_wq"]).astype(np.float64)
        wk1 = np.asarray(inputs["attn1_wk"]).astype(np.float64)
        wv1 = np.asarray(inputs["attn1_wv"]).astype(np.float64)
        wo1 = np.asarray(inputs["attn1_wo"]).astype(np.float64)
        s1, h1 = e[1]
        m["a1_wqT"] = wib(wq1 * s1[:, None])
        m["a1_wkT"] = np.ascontiguousarray(
            wib(wk1 * s1[:, None]).transpose(1, 0, 2, 3))
        m["a1_wv"] = bf((wv1 * s1[:, None]).reshape(DB, P, INNER))
        m["a1_wo"] = bf(wo1.reshape(NPAIR, P, D))
        m["bq1"] = f32((h1 @ wq1).reshape(DB, P).T)
        bo1 = (np.asarray(inputs["attn1_bo"]).astype(np.float64)
               + (h1 @ wv1) @ wo1)                # V-shift folded through wo
        wq2 = np.asarray(inputs["attn2_wq"]).astype(np.float64)
        s2, h2 = e[2]
        m["a2_wqT"] = wib(wq2 * s2[:, None])
        m["a2_wkT"] = wib(np.asarray(inputs["attn2_wk"]))
        m["a2_wv"] = bf(np.asarray(inputs["attn2_wv"]).reshape(DB, P, INNER))
        m["a2_wo"] = bf(np.asarray(inputs["attn2_wo"]).reshape(NPAIR, P, D))
        m["bq2"] = f32((h2 @ wq2).reshape(DB, P).T)
        m["bo2"] = bf(np.asarray(inputs["attn2_bo"]).reshape(1, D))
        w1 = np.asarray(inputs["ff_w1"]).astype(np.float64)
        s3, h3 = e[3]
        m["w1"] = np.ascontiguousarray(
            (w1 * s3[:, None]).reshape(DB, P, 64, P)
            .transpose(2, 1, 0, 3).astype(NPBF16))
        b1 = np.asarray(inputs["ff_b1"]).astype(np.float64) + h3 @ w1
        m["b1a"] = f32(b1[:DFF].reshape(32, P).T)
        m["b1g"] = f32(b1[DFF:].reshape(32, P).T)
        m["w2"] = bf(np.asarray(inputs["ff_w2"]).reshape(32, P, D))
        m["b2"] = bf(np.asarray(inputs["ff_b2"]).reshape(1, D))
        m["ctx"] = bf(context[b])
        m["_bo1"] = bo1
        per_batch.append(m)

    in_maps = []
    for c in range(NCORES):
        b, q = c // 4, c % 4
        pb = per_batch[b]
        m = {k: v for k, v in pb.items() if not k.startswith("_")}
        m["x_bf"] = bf(np.roll(x[b], -q * OWN, axis=0))
        m["x_own"] = f32(x[b, q * OWN:(q + 1) * OWN].astype(np.float64)
                         + pb["_bo1"][None, :])
        in_maps.append(m)
    return in_maps


_CACHE = {}


def kernel(**inputs):
    if "nc" not in _CACHE:
        _CACHE["nc"] = build_program()
    nc = _CACHE["nc"]
    key = tuple(id(inputs[k]) for k in sorted(inputs))
    if _CACHE.get("prep_key") != key:
        _CACHE["in_maps"] = host_prep(inputs)
        _CACHE["prep_key"] = key
        _CACHE["prep_refs"] = inputs
    in_maps = _CACHE["in_maps"]
    res = bass_utils.run_bass_kernel_spmd(
        nc, in_maps, core_ids=list(range(NCORES)), trace=False)
    _CACHE["last_exec_ns"] = res.exec_time_ns
    _CACHE["last_results"] = res
    out = np.empty((B, S, D), np.float32)
    for c in range(NCORES):
        b, q = c // 4, c % 4
        out[b, q * OWN:(q + 1) * OWN] = res.results[c]["out"]
    return out
